# revision 6
# baseline (speedup 1.0000x reference)
"""Titans NeuralMemory forward on 8 Trainium2 NeuronCores.

Decomposition (validated vs reference in fp64/numpy):
  - Per-chunk MLP-loss gradients are rank-16: g_i(s) = l_i(s)^T r_i(s) with
    l/r factors [16, 256] computed from a batched forward/backward pass with
    the shared base weights.
  - The two associative scans (momentum, updates) have scalar per-chunk
    coefficients, so their composition is a lower-triangular [64, 64] matrix
    T = L_D @ L_A built stably via exp of cumulative log-sigmoid differences.
  - Retrieval never materializes fast weights: per layer,
      X_{i+1} = silu(X_i @ W_i + (X_i @ L_i^T * M) @ R_i),
    where M[r, j] = T[chunk(r), chunk(j)] expands T blockwise.

Sharding: 8 cores = 2 batch rows x 4 retrieve row-groups of 256 rows.
Each core redundantly runs the store phase (factors for all 64 chunks of its
batch row) and computes its own 256 retrieve rows; no collectives.
All matmuls run in fp32r (full PE rate); PSUM accumulation is fp32.
"""
import os
import numpy as np

import concourse.bass as bass
import concourse.tile as tile
from concourse import bacc, mybir
from concourse.bass_utils import run_bass_kernel_spmd

AF = mybir.ActivationFunctionType
ALU = mybir.AluOpType
FP32 = mybir.dt.float32
FP32R = mybir.dt.float32r

B, L, D, C, DEPTH = 2, 1024, 256, 16, 4
N = L // C          # 64 chunks
P = 128
EPS = 1.1920929e-07
NCORES = 8
GROUPS = 4          # retrieve row-groups per batch row
RT = L // GROUPS    # 256 retrieve rows per core

_CACHE = {}
LAST_PERF = {}


def _install_ntff_hook():
    """The agent image's antenv lacks axon_hooks; synthesize it so
    run_bass_kernel_spmd's trace=True path can reach the NTFF ctypes hook."""
    import sys
    import types
    try:
        from trn_agent_boot.trn_boot import _ntff_profile_via_ctypes
        hook = _ntff_profile_via_ctypes("/opt/axon/libaxon_pjrt.so")
    except Exception:
        return False
    if hook is None:
        return False
    mod = types.ModuleType("antenv.axon_hooks")
    mod.get_axon_ntff_profile_hook = lambda: hook
    mod.set_axon_ntff_profile_hook = lambda h: None
    sys.modules["antenv.axon_hooks"] = mod
    return True


def _rmsnorm_tiles(nc, pool, src_ap, n_tiles, tag, eps_sb):
    """Load+normalize [n_tiles*128, 256] from DRAM; returns list of fp32 tiles."""
    outs = []
    for i in range(n_tiles):
        x = pool.tile([P, D], FP32, tag=f"{tag}x{i}", bufs=1)
        nc.sync.dma_start(x, src_ap[i * P:(i + 1) * P, :])
        x2 = pool.tile([P, D], FP32, tag=f"{tag}sq", bufs=2)
        nc.vector.tensor_mul(x2, x, x)
        ms = pool.tile([P, 1], FP32, tag=f"{tag}ms", bufs=2)
        nc.vector.reduce_sum(ms, x2, axis=mybir.AxisListType.X)
        lnv = pool.tile([P, 1], FP32, tag=f"{tag}ln", bufs=2)
        nc.scalar.activation(lnv, ms, AF.Ln, scale=1.0 / D, bias=eps_sb)
        rstd = pool.tile([P, 1], FP32, tag=f"{tag}rs", bufs=2)
        nc.scalar.activation(rstd, lnv, AF.Exp, scale=-0.5)
        nc.vector.tensor_scalar_mul(x, x, rstd)
        outs.append(x)
    return outs


def _build():
    nc = bacc.Bacc("TRN2", target_bir_lowering=False)

    # ---------------- DRAM I/O ----------------
    seq_b = nc.dram_tensor("seq_b", [L, D], FP32, kind="ExternalInput")
    seq_q = nc.dram_tensor("seq_q", [RT, D], FP32, kind="ExternalInput")
    wq_d = nc.dram_tensor("wq_d", [P, 2, D], FP32R, kind="ExternalInput")
    wkv_d = nc.dram_tensor("wkv_d", [P, 2, 2 * D], FP32R, kind="ExternalInput")
    w_d = nc.dram_tensor("w_d", [P, 4, 2, D], FP32R, kind="ExternalInput")
    wt_d = nc.dram_tensor("wt_d", [P, 3, 2, D], FP32R, kind="ExternalInput")
    wpack_d = nc.dram_tensor("wpack_d", [P, 2, 4], FP32R, kind="ExternalInput")
    ident_d = nc.dram_tensor("ident_d", [P, P], FP32, kind="ExternalInput")
    ones_ut_d = nc.dram_tensor("ones_ut_d", [P, P], FP32R, kind="ExternalInput")
    neg_ut_d = nc.dram_tensor("neg_ut_d", [P, P], FP32R, kind="ExternalInput")
    mask_ls_d = nc.dram_tensor("mask_ls_d", [P, N], FP32, kind="ExternalInput")
    mask_ut_d = nc.dram_tensor("mask_ut_d", [P, N], FP32, kind="ExternalInput")
    sel_d = nc.dram_tensor("sel_d", [P, C], FP32R, kind="ExternalInput")
    out_d = nc.dram_tensor("out", [RT, D], FP32, kind="ExternalOutput")

    with tile.TileContext(nc) as tc:
        with (
            tc.tile_pool(name="big", bufs=1) as big,
            tc.tile_pool(name="rot", bufs=3) as rot,
            tc.tile_pool(name="pmm", bufs=2, space="PSUM") as pmm,
            tc.tile_pool(name="psc", bufs=2, space="PSUM") as psc,
            tc.tile_pool(name="ptr", bufs=2, space="PSUM") as ptr,
            tc.tile_pool(name="dram", bufs=1, space="DRAM") as dram,
        ):
            # ---------------- load constants/weights ----------------
            eps_sb = big.tile([P, 1], FP32)
            nc.vector.memset(eps_sb, EPS)
            identF = big.tile([P, P], FP32)
            nc.sync.dma_start(identF, ident_d[:])
            ut_sb = big.tile([P, P], FP32R)
            nc.sync.dma_start(ut_sb, ones_ut_d[:])
            nut_sb = big.tile([P, P], FP32R)
            nc.sync.dma_start(nut_sb, neg_ut_d[:])
            mls_sb = big.tile([P, N], FP32)
            nc.sync.dma_start(mls_sb, mask_ls_d[:])
            mut_sb = big.tile([P, N], FP32)
            nc.sync.dma_start(mut_sb, mask_ut_d[:])
            sel_sb = big.tile([P, C], FP32R)
            nc.sync.dma_start(sel_sb, sel_d[:])
            wp_sb = big.tile([P, 2, 4], FP32R)
            nc.sync.dma_start(wp_sb, wpack_d[:])
            wq_sb = big.tile([P, 2, D], FP32R)
            nc.sync.dma_start(wq_sb, wq_d[:])
            wkv_sb = big.tile([P, 2, 2 * D], FP32R)
            nc.sync.dma_start(wkv_sb, wkv_d[:])
            w_sb = big.tile([P, 4, 2, D], FP32R)
            nc.sync.dma_start(w_sb, w_d[:])
            wt_sb = big.tile([P, 3, 2, D], FP32R)
            nc.sync.dma_start(wt_sb, wt_d[:])

            # ---------------- store rmsnorm + transpose ----------------
            sn = _rmsnorm_tiles(nc, rot, seq_b[:], 8, "sn", eps_sb)
            snT = [big.tile([P, L], FP32R, name=f"snT{k}") for k in range(2)]
            for i in range(8):
                for ko in range(2):
                    tr = ptr.tile([P, P], FP32, tag="tr")
                    nc.tensor.transpose(tr, sn[i][:, ko * P:(ko + 1) * P], identF)
                    nc.any.tensor_copy(snT[ko][:, i * P:(i + 1) * P], tr)

            # ---------------- chunk means (sums; 1/16 folded into wpack) ----
            cmT = big.tile([P, 2, N], FP32R)
            with nc.allow_low_precision(reason="fp32r rounding of fp32 accum"):
                for ko in range(2):
                    nc.vector.reduce_sum(
                        cmT[:, ko, :],
                        snT[ko].rearrange("p (n c) -> p n c", c=C),
                        axis=mybir.AxisListType.X)

            # ---------------- T-matrix pipeline ----------------
            zp = ptr.tile([N, 4], FP32, tag="tr")
            for ko in range(2):
                nc.tensor.matmul(zp, cmT[:, ko, :], wp_sb[:, ko, :],
                                 start=(ko == 0), stop=(ko == 1))
            # lrs = sigmoid(z0) * (-2/D)
            lrs = big.tile([N, 1], FP32)
            nc.scalar.activation(lrs, zp[:, 0:1], AF.Sigmoid)
            nc.vector.tensor_scalar_mul(lrs, lrs, -2.0 / D)
            # lg = [ln(sigmoid(z1)), ln(sigmoid(-z2))]
            sg = big.tile([P, 2], FP32)
            nc.vector.memset(sg, 0.0)
            nc.scalar.activation(sg[:N, 0:1], zp[:, 1:2], AF.Sigmoid)
            nc.scalar.activation(sg[:N, 1:2], zp[:, 2:3], AF.Sigmoid, scale=-1.0)
            lg = big.tile([P, 2], FP32)
            nc.vector.memset(lg, 0.0)
            nc.scalar.activation(lg[:N, :], sg[:N, :], AF.Ln)
            lgr = big.tile([P, 2], FP32R)
            nc.vector.tensor_copy(lgr, lg)
            cacc_p = ptr.tile([P, 2], FP32, tag="tr")
            nc.tensor.matmul(cacc_p, ut_sb, lgr, start=True, stop=True)
            cacc = big.tile([P, 2], FP32)
            nc.vector.tensor_copy(cacc, cacc_p)
            nacc_p = ptr.tile([P, 2], FP32, tag="tr")
            nc.tensor.matmul(nacc_p, nut_sb, lgr, start=True, stop=True)
            nacc = big.tile([P, 2], FP32)
            nc.vector.tensor_copy(nacc, nacc_p)

            # stage [NACC0 | CACC1 | lrs] -> DRAM -> row broadcasts
            stage = big.tile([P, 3], FP32)
            nc.vector.tensor_copy(stage[:, 0:1], nacc[:, 0:1])
            nc.vector.tensor_copy(stage[:, 1:2], cacc[:, 1:2])
            nc.vector.memset(stage[:, 2:3], 0.0)
            nc.vector.tensor_copy(stage[:N, 2:3], lrs)
            scr = dram.tile([P, 3], FP32)
            nc.sync.dma_start(scr, stage)
            ncarow = big.tile([P, N], FP32)
            nc.sync.dma_start(ncarow, bass.AP(
                tensor=scr.tensor, offset=scr.offset, ap=[[0, P], [3, N]]))
            pcdrow = big.tile([P, N], FP32)
            nc.sync.dma_start(pcdrow, bass.AP(
                tensor=scr.tensor, offset=scr.offset + 1, ap=[[0, P], [3, N]]))
            lrb = big.tile([P, N], FP32)
            nc.sync.dma_start(lrb, bass.AP(
                tensor=scr.tensor, offset=scr.offset + 2, ap=[[0, P], [3, N]]))

            # L_A[t,s] = exp(CA_t - CA_s) (t>=s), LDT[r,t] = exp(CD_t - CD_r) (r<=t)
            la = big.tile([P, N], FP32R)
            tmp1 = big.tile([P, N], FP32)
            nc.vector.scalar_tensor_tensor(
                out=tmp1, in0=ncarow, scalar=cacc[:, 0:1], in1=mls_sb,
                op0=ALU.add, op1=ALU.add)
            nc.scalar.activation(la, tmp1, AF.Exp)
            ldt = big.tile([P, N], FP32R)
            tmp2 = big.tile([P, N], FP32)
            nc.vector.scalar_tensor_tensor(
                out=tmp2, in0=pcdrow, scalar=nacc[:, 1:2], in1=mut_sb,
                op0=ALU.add, op1=ALU.add)
            nc.scalar.activation(ldt, tmp2, AF.Exp)

            # Ttile[t,s] = sum_r LDT[r,t] L_A[r,s]
            tt_p = ptr.tile([N, N], FP32, tag="tr")
            nc.tensor.matmul(tt_p, ldt, la, start=True, stop=True)
            ttile = big.tile([P, N], FP32)
            nc.vector.memset(ttile, 0.0)
            nc.vector.tensor_copy(ttile[:N], tt_p)

            # maskb_k[j, i] = T[toff+i, s(j)] via sel matmul
            maskb = []
            for k in range(8):
                ttx = rot.tile([P, P], FP32R, tag="ttx", bufs=2)
                nc.vector.tensor_copy(
                    ttx[:N],
                    ttile[:N, k * 8:(k + 1) * 8, None].to_broadcast([N, 8, C]))
                mb_p = ptr.tile([P, C], FP32, tag="tr")
                nc.tensor.matmul(mb_p, ttx[:N], sel_sb[:N], start=True, stop=True)
                mb = big.tile([P, C], FP32, name=f"maskb{k}")
                nc.any.tensor_copy(mb, mb_p)
                maskb.append(mb)

            # ---------------- kv projection ----------------
            kT = [big.tile([P, L], FP32R, name=f"kT{k}") for k in range(2)]
            vT = [big.tile([P, L], FP32, name=f"vT{k}") for k in range(2)]
            for ko4 in range(4):
                dest = kT[ko4] if ko4 < 2 else vT[ko4 - 2]
                for rc in range(2):
                    mm = pmm.tile([P, 512], FP32, tag="mm")
                    for ki in range(2):
                        nc.tensor.matmul(
                            mm, wkv_sb[:, ki, ko4 * P:(ko4 + 1) * P],
                            snT[ki][:, rc * 512:(rc + 1) * 512],
                            start=(ki == 0), stop=(ki == 1))
                    nc.any.tensor_copy(dest[:, rc * 512:(rc + 1) * 512], mm)

            # ---------------- forward MLP (base weights) ----------------
            Lf = [kT]
            dsT = []
            for i in range(3):
                a_next = [big.tile([P, L], FP32R, name=f"aT{i+1}_{k}")
                          for k in range(2)]
                ds_i = [big.tile([P, L], FP32, name=f"dsT{i}_{k}")
                        for k in range(2)]
                for mo in range(2):
                    for rc in range(2):
                        sl = slice(rc * 512, (rc + 1) * 512)
                        mm = pmm.tile([P, 512], FP32, tag="mm")
                        for ki in range(2):
                            nc.tensor.matmul(
                                mm, w_sb[:, i, ki, mo * P:(mo + 1) * P],
                                Lf[i][ki][:, sl],
                                start=(ki == 0), stop=(ki == 1))
                        sgt = rot.tile([P, 512], FP32, tag="sgt")
                        nc.scalar.activation(sgt, mm, AF.Sigmoid)
                        nc.vector.tensor_mul(a_next[mo][:, sl], mm, sgt)
                        t2 = rot.tile([P, 512], FP32, tag="t2")
                        nc.vector.scalar_tensor_tensor(
                            out=t2, in0=mm, scalar=1.0, in1=a_next[mo][:, sl],
                            op0=ALU.add, op1=ALU.subtract)
                        nc.vector.tensor_mul(ds_i[mo][:, sl], sgt, t2)
                Lf.append(a_next)
                dsT.append(ds_i)

            # ---------------- pred + gg3 ----------------
            ggA = [big.tile([P, L], FP32R, name=f"ggA{k}") for k in range(2)]
            ggB = [big.tile([P, L], FP32R, name=f"ggB{k}") for k in range(2)]
            for mo in range(2):
                for rc in range(2):
                    sl = slice(rc * 512, (rc + 1) * 512)
                    mm = pmm.tile([P, 512], FP32, tag="mm")
                    for ki in range(2):
                        nc.tensor.matmul(
                            mm, w_sb[:, 3, ki, mo * P:(mo + 1) * P],
                            Lf[3][ki][:, sl],
                            start=(ki == 0), stop=(ki == 1))
                    d = rot.tile([P, 512], FP32, tag="d")
                    nc.vector.tensor_sub(d, mm, vT[mo][:, sl])
                    nc.vector.tensor_tensor(
                        ggA[mo][:, sl].rearrange("p (n c) -> p n c", c=C),
                        d.rearrange("p (n c) -> p n c", c=C),
                        lrb[:, rc * 32:(rc + 1) * 32, None]
                        .to_broadcast([P, 32, C]),
                        ALU.mult)

            # ---------------- R factors + backward chain ----------------
            Rf = {i: [big.tile([P, D], FP32R, name=f"Rf{i}_{jt}")
                      for jt in range(8)] for i in range(4)}

            def emit_R(layer, src):
                for jt in range(8):
                    for mo in range(2):
                        tr = ptr.tile([P, P], FP32, tag="tr")
                        nc.tensor.transpose(
                            tr, src[mo][:, jt * P:(jt + 1) * P].bitcast(FP32),
                            identF)
                        nc.any.tensor_copy(
                            Rf[layer][jt][:, mo * P:(mo + 1) * P], tr)

            emit_R(3, ggA)
            gg_cur = ggA
            gg_next = ggB
            for i in (3, 2, 1):
                for mo in range(2):
                    for rc in range(2):
                        sl = slice(rc * 512, (rc + 1) * 512)
                        mm = pmm.tile([P, 512], FP32, tag="mm")
                        for ki in range(2):
                            nc.tensor.matmul(
                                mm, wt_sb[:, i - 1, ki, mo * P:(mo + 1) * P],
                                gg_cur[ki][:, sl],
                                start=(ki == 0), stop=(ki == 1))
                        nc.vector.tensor_mul(
                            gg_next[mo][:, sl], mm, dsT[i - 1][mo][:, sl])
                emit_R(i - 1, gg_next)
                gg_cur, gg_next = gg_next, gg_cur

            # ---------------- retrieve: q projection ----------------
            rq = _rmsnorm_tiles(nc, rot, seq_q[:], 2, "rq", eps_sb)
            rqT = [big.tile([P, RT], FP32R, name=f"rqT{k}") for k in range(2)]
            for rt in range(2):
                for ko in range(2):
                    tr = ptr.tile([P, P], FP32, tag="tr")
                    nc.tensor.transpose(tr, rq[rt][:, ko * P:(ko + 1) * P], identF)
                    nc.any.tensor_copy(rqT[ko][:, rt * P:(rt + 1) * P], tr)

            XTa = [big.tile([P, RT], FP32R, name=f"XTa{k}") for k in range(2)]
            XTb = [big.tile([P, RT], FP32R, name=f"XTb{k}") for k in range(2)]
            for mo in range(2):
                sc = psc.tile([P, RT], FP32, tag="sc")
                for ki in range(2):
                    nc.tensor.matmul(sc, wq_sb[:, ki, mo * P:(mo + 1) * P],
                                     rqT[ki], start=(ki == 0), stop=(ki == 1))
                nc.any.tensor_copy(XTa[mo], sc)

            # ---------------- retrieve layers ----------------
            XTin = XTa
            XTout = XTb
            X4T = [big.tile([P, RT], FP32, name=f"X4T{k}") for k in range(2)]
            for i in range(4):
                msc = []
                for jt in range(8):
                    sc = psc.tile([P, RT], FP32, tag="sc")
                    for ki in range(2):
                        nc.tensor.matmul(
                            sc, Lf[i][ki][:, jt * P:(jt + 1) * P], XTin[ki],
                            start=(ki == 0), stop=(ki == 1))
                    m = rot.tile([P, RT], FP32R, tag="msc", bufs=9)
                    nc.vector.tensor_tensor(
                        m.rearrange("p (n c) -> p n c", c=C),
                        sc.rearrange("p (n c) -> p n c", c=C),
                        maskb[jt][:, :, None].to_broadcast([P, C, C]),
                        ALU.mult)
                    msc.append(m)
                for mo in range(2):
                    y = psc.tile([P, RT], FP32, tag="y")
                    for ki in range(2):
                        nc.tensor.matmul(
                            y, w_sb[:, i, ki, mo * P:(mo + 1) * P], XTin[ki],
                            start=(ki == 0), stop=False)
                    for jt in range(8):
                        nc.tensor.matmul(
                            y, Rf[i][jt][:, mo * P:(mo + 1) * P], msc[jt],
                            start=False, stop=(jt == 7))
                    if i < 3:
                        sgt = rot.tile([P, RT], FP32, tag="sgr")
                        nc.scalar.activation(sgt, y, AF.Sigmoid)
                        nc.vector.tensor_mul(XTout[mo], y, sgt)
                    else:
                        nc.any.tensor_copy(X4T[mo], y)
                XTin, XTout = XTout, XTin

            # ---------------- postnorm + output ----------------
            for rt in range(2):
                x4 = rot.tile([P, D], FP32, tag="x4", bufs=2)
                for mo in range(2):
                    tr = ptr.tile([P, P], FP32, tag="tr")
                    nc.tensor.transpose(tr, X4T[mo][:, rt * P:(rt + 1) * P],
                                        identF)
                    nc.any.tensor_copy(x4[:, mo * P:(mo + 1) * P], tr)
                x2 = rot.tile([P, D], FP32, tag="px2", bufs=2)
                nc.vector.tensor_mul(x2, x4, x4)
                ms = rot.tile([P, 1], FP32, tag="pms", bufs=2)
                nc.vector.reduce_sum(ms, x2, axis=mybir.AxisListType.X)
                lnv = rot.tile([P, 1], FP32, tag="pln", bufs=2)
                nc.scalar.activation(lnv, ms, AF.Ln, scale=1.0 / D, bias=eps_sb)
                rstd = rot.tile([P, 1], FP32, tag="prs", bufs=2)
                nc.scalar.activation(rstd, lnv, AF.Exp, scale=-0.5)
                o = rot.tile([P, D], FP32, tag="osb", bufs=2)
                nc.vector.tensor_scalar_mul(o, x4, rstd)
                nc.sync.dma_start(out_d[rt * P:(rt + 1) * P, :], o)

    nc.compile()
    return nc


def _host_prep(inputs):
    seq = np.ascontiguousarray(np.asarray(inputs["seq"], dtype=np.float32))
    Wq = np.asarray(inputs["Wq"], dtype=np.float32)
    Wkv = np.asarray(inputs["Wkv"], dtype=np.float32)
    Ws = [np.asarray(inputs[f"W{i}"], dtype=np.float32) for i in range(4)]
    wa = np.asarray(inputs["w_adapt"], dtype=np.float32)
    wm = np.asarray(inputs["w_mom"], dtype=np.float32)
    wd = np.asarray(inputs["w_decay"], dtype=np.float32)

    def kxm(w):  # [K, M] -> [128, K/128, M]
        return np.ascontiguousarray(
            w.reshape(w.shape[0] // P, P, w.shape[1]).transpose(1, 0, 2))

    shared = {}
    shared["wq_d"] = kxm(Wq)
    shared["wkv_d"] = kxm(Wkv)
    w_all = np.stack(Ws)  # [4, 256, 256]
    shared["w_d"] = np.ascontiguousarray(
        w_all.reshape(4, 2, P, D).transpose(2, 0, 1, 3))
    wt_all = np.stack([Ws[1].T, Ws[2].T, Ws[3].T])
    shared["wt_d"] = np.ascontiguousarray(
        wt_all.reshape(3, 2, P, D).transpose(2, 0, 1, 3))
    wpack = np.zeros((D, 4), np.float32)
    wpack[:, 0] = wa
    wpack[:, 1] = wm
    wpack[:, 2] = wd
    wpack *= (1.0 / C)  # chunk-mean folded (device computes chunk sums)
    shared["wpack_d"] = kxm(wpack)
    shared["ident_d"] = np.eye(P, dtype=np.float32)
    tri = np.triu(np.ones((N, N), np.float32))
    ones_ut = np.zeros((P, P), np.float32)
    ones_ut[:N, :N] = tri
    shared["ones_ut_d"] = ones_ut
    shared["neg_ut_d"] = -ones_ut
    ii = np.arange(N)
    mask_ls = np.full((P, N), -1e30, np.float32)
    mask_ls[:N] = np.where(ii[:, None] >= ii[None, :], 0.0, -1e30)
    shared["mask_ls_d"] = mask_ls
    mask_ut = np.full((P, N), -1e30, np.float32)
    mask_ut[:N] = np.where(ii[:, None] <= ii[None, :], 0.0, -1e30)
    shared["mask_ut_d"] = mask_ut

    in_maps = []
    for core in range(NCORES):
        b, g = divmod(core, GROUPS)
        m = dict(shared)
        m["seq_b"] = seq[b]
        qs = np.zeros((RT, D), np.float32)
        j0 = RT * g + (C - 1)
        src = seq[b, j0:min(j0 + RT, L)]
        qs[:len(src)] = src
        m["seq_q"] = qs
        sel = np.zeros((P, C), np.float32)
        toff = C * g
        sel[toff:toff + C, :] = np.eye(C, dtype=np.float32)
        m["sel_d"] = sel
        in_maps.append(m)
    return in_maps


def kernel(**inputs):
    if "nc" not in _CACHE:
        _CACHE["nc"] = _build()
    nc = _CACHE["nc"]
    in_maps = _host_prep(inputs)
    trace = bool(int(os.environ.get("KERNEL_TRACE", "0")))
    if trace:
        try:
            from antenv.axon_hooks import get_axon_ntff_profile_hook  # noqa: F401
        except ImportError:
            trace = _install_ntff_hook()
    res = run_bass_kernel_spmd(
        nc, in_maps, core_ids=list(range(NCORES)), trace=trace)
    LAST_PERF.clear()
    LAST_PERF.update(dict(
        exec_time_ns=res.exec_time_ns,
        mean_exec_time_ns=res.mean_exec_time_ns,
        trace=res.instructions_and_trace[1] if res.instructions_and_trace else None,
    ))
    final = np.zeros((B, L, D), np.float32)
    for core in range(NCORES):
        b, g = divmod(core, GROUPS)
        j0 = RT * g + (C - 1)
        n = min(RT, L - j0)
        final[b, j0:j0 + n] = res.results[core]["out"][:n]
    return final
